# revision 12
# baseline (speedup 1.0000x reference)
"""Int8RouterLinear TRN2 kernel: out[16384, 64] = x[16384, 4096] @ (W_int8 * scale)^T.

v2 strategy (data-parallel over 8 NeuronCores, 2048 tokens each):
  - Host quantizes x per token: h-tiles k>=8 to int8 (u = rint(x/s_t),
    s_t = absmax_t/127), h-tiles k<8 to fp8-e4m3 of x/s_t. 1 byte/elem
    either way -> 8MB of x per core (vs 14.1MB for the fp16/fp8 mix).
    int8's uniform grid is ~3x more accurate than fp8 for Gaussian x.
  - On device, int8 h-tiles are cast to fp16 (exact: |u| <= 127) split
    across DVE (2x mode, ~1.92 elem/ns/partition) and ACT
    ((N+352)/1.2ns); fp8 tiles feed the PE directly (fp16 lhsT x fp8
    rhs mixed matmul, same speed).
  - PE runs col-tiled: the 2048 tokens form 2 super-chunks of 1024; a
    super-chunk's two 512-token halves run CONCURRENTLY in PE column
    groups 0-63 / 64-127 (tile_position via out base partition), so a
    k-step costs ~216ns for 1024 tokens -> ~14us PE total.
  - PSUM: one [128, 512] f32 bank per super-chunk (half-partitions =
    token halves), accumulated over the 32 h-tiles, then one ACT
    scaled-copy (2^-6, fits fp16) -> [128, 512] fp16 out, DMA'd out.
  - Host post-scales: out = psum_fp16 * 2^6 * s_t * scale_e. Weight
    ships as fp16 (int8 values exact).
  - DMA: x + w + out = 8.75MB/core over both HWDGE rings, blocks
    interleaved in program (k) order so completion tracks the
    cast/matmul consumption order.
"""
import numpy as np

import concourse.mybir as mybir
from concourse import bacc
from concourse.tile import TileContext
from concourse.bass_utils import run_bass_kernel_spmd

TOKENS = 16384
HIDDEN = 4096
EXPERTS = 64
NCORES = 8
TSHARD = TOKENS // NCORES          # 2048 tokens per core
HT = HIDDEN // 128                 # 32 h-tiles of 128
HT8 = 8                            # leading h-tiles in fp8 (no cast)
HTI = HT - HT8                     # trailing h-tiles shipped as int8
NS = 2                             # super-chunks of 1024 tokens
SU = 1024                          # tokens per super-chunk
CH = 512                           # tokens per col-group chunk

F32 = mybir.dt.float32
F16 = mybir.dt.float16
F8 = mybir.dt.float8e4
I8 = mybir.dt.int8

# DMA blocks: (name, ring, kind, u0, nu) with u0 an ABSOLUTE unit index
# into x8_d (f8: s*8+k) or xi_d (i8: s*24+(k-8)); i8 blocks may cross
# the super-chunk boundary. Constraints learned from traces:
#  - HWDGE ring depth ~4 in-flight transfers; dispatch #5+ waits a
#    completion, so blocks must be big enough to keep the wire fed.
#  - Ring1 dispatches share the ACT sequencer FIFO with ACT casts: ring1
#    gets ONLY 4 upfront transfers and nothing mid-stream.
#  - Tiny tail blocks so the last receipt gates minimal work.
BLOCKS = [
    ("a0",  0, "i8", 0, 2),    # s0 k8-9 (DVE start)
    ("w1",  1, "w",  0, 0),
    ("b1",  1, "f8", 0, 8),    # f8 s0 (PE start)
    ("c0",  0, "i8", 2, 8),    # s0 k10-17
    ("d1",  1, "i8", 10, 8),   # s0 k18-25
    ("e0",  0, "i8", 18, 8),   # s0 k26-31 + s1 k8-9
    ("g1",  1, "i8", 26, 8),   # s1 k10-17
    ("f0",  0, "f8", 8, 8),    # f8 s1
    ("h1",  1, "i8", 34, 4),   # s1 k18-21 (ring1: balances ring bytes)
    ("i0",  0, "i8", 38, 4),   # s1 k22-25
    ("j0",  0, "i8", 42, 2),   # s1 k26-27
    ("k0",  0, "i8", 44, 2),   # s1 k28-29
    ("l1",  1, "i8", 46, 1),   # s1 k30 (rides ring1 early, consumed last)
    ("m1",  1, "i8", 47, 1),   # s1 k31
]
# int8 cast ops: (engine, s, k0, nk) in consumption order; each op reads
# within one landed block. DVE 32u@2x + w + s1 tail copy; ACT 16u@1x +
# s0 tail copy. "MID" marks where s0's MMs + ACT tail copy are emitted
# (by then s0's MMs have no pending deps -> no head-of-line stall).
CASTS = [
    ("v", 0, 8, 2),
    ("v", 0, 10, 5), ("a", 0, 15, 3),
    ("v", 0, 18, 5), ("a", 0, 23, 3),
    ("v", 0, 26, 4), ("a", 0, 30, 2), ("v", 1, 8, 2),
    ("v", 1, 10, 5), ("a", 1, 15, 3),
    ("MID", 0, 0, 0),
    ("v", 1, 18, 3), ("a", 1, 21, 1),
    ("v", 1, 22, 2), ("a", 1, 24, 2),
    ("v", 1, 26, 1), ("a", 1, 27, 1),
    ("v", 1, 28, 1), ("a", 1, 29, 1),
    ("v", 1, 30, 1), ("v", 1, 31, 1),
]

_cache = {}


def _build():
    if "nc" in _cache:
        return _cache["nc"]

    nc = bacc.Bacc("TRN2", target_bir_lowering=False, debug=False,
                   num_devices=NCORES)
    x8_d = nc.dram_tensor("x8", [128, NS * HT8, SU], F8, kind="ExternalInput")
    xi_d = nc.dram_tensor("xi", [128, NS * HTI, SU], I8, kind="ExternalInput")
    w_d = nc.dram_tensor("w", [128, HT * EXPERTS], I8, kind="ExternalInput")
    o_d = nc.dram_tensor("out", [128, NS * CH], F16, kind="ExternalOutput")

    with TileContext(nc) as tc:
        with tc.tile_pool(name="consts", bufs=1) as cpool, \
             tc.tile_pool(name="xp", bufs=1) as xpool, \
             tc.tile_pool(name="xf", bufs=1) as fpool, \
             tc.tile_pool(name="pso", bufs=1, space="PSUM") as ppool, \
             tc.tile_pool(name="ost", bufs=1) as opool:
            rings = [nc.sync, nc.scalar]

            src_tiles = {}      # (kind, absolute unit) -> (tile, j)
            w_i8 = cpool.tile([128, HT * EXPERTS], I8)
            w_sb = cpool.tile([128, HT * EXPERTS], F16)

            for name, ring, kind, u0, nu in BLOCKS:
                if kind == "w":
                    rings[ring].dma_start(out=w_i8, in_=w_d.ap())
                    continue
                src_d = x8_d if kind == "f8" else xi_d
                dt = F8 if kind == "f8" else I8
                t = xpool.tile([128, nu * SU], dt, name=name, tag=name)
                rings[ring].dma_start(out=t, in_=src_d.ap()[:, u0:u0 + nu, :])
                for j in range(nu):
                    src_tiles[(kind, u0 + j)] = (t, j)

            def unit(s, k):
                if k < HT8:
                    return src_tiles[("f8", s * HT8 + k)]
                return src_tiles[("i8", s * HTI + (k - HT8))]

            nc.vector.tensor_copy(w_sb, w_i8)   # DVE's first op, ~1.1us
            w_v = w_sb.rearrange("p (k e) -> p k e", e=EXPERTS)

            pps = [ppool.tile([128, CH], F32, name=f"pp{s}", tag=f"pp{s}")
                   for s in range(NS)]
            ots = [opool.tile([128, CH], F16, name=f"ot{s}", tag=f"ot{s}")
                   for s in range(NS)]

            def mm_superchunk(s):
                pp = pps[s]
                for k in range(HT):
                    t, j = unit(s, k) if k < HT8 else f16_units[(s, k)]
                    tv = t.rearrange("p (u t) -> p u t", t=SU)
                    wt = w_v[:, k, :]
                    nc.tensor.matmul(pp[0:64, :], wt, tv[:, j, 0:CH],
                                     start=(k == 0), stop=(k == HT - 1))
                    nc.tensor.matmul(pp[64:128, :], wt, tv[:, j, CH:SU],
                                     start=(k == 0), stop=(k == HT - 1))
                # tail: one scaled fp32->fp16 copy (2^-6 keeps |v| < 2^16/6.4)
                # s0 on ACT (inserted mid-FIFO), s1 on DVE (idle at the end).
                if s == 0:
                    nc.scalar.mul(ots[s], pp, 0.015625)
                else:
                    nc.vector.tensor_scalar_mul(ots[s], pp, 0.015625)
                rings[0].dma_start(out=o_d.ap()[:, s * CH:(s + 1) * CH],
                                   in_=ots[s])

            # cast int8 units -> fp16 tiles; at MID, emit all of s0's MMs
            # + its ACT tail copy so the copy sits early in the ACT FIFO.
            f16_units = {}
            for eng, s, k0, nk in CASTS:
                if eng == "MID":
                    mm_superchunk(0)
                    continue
                it, j0 = unit(s, k0)
                iv = it.rearrange("p (u t) -> p u t", t=SU)
                ft = fpool.tile([128, nk * SU], F16, name=f"c{eng}{s}k{k0}",
                                tag=f"c{eng}{s}k{k0}")
                if eng == "v":
                    nc.vector.tensor_copy(ft, iv[:, j0:j0 + nk, :])
                else:
                    nc.scalar.copy(ft, iv[:, j0:j0 + nk, :])
                for j in range(nk):
                    f16_units[(s, k0 + j)] = (ft, j)
            mm_superchunk(1)

    nc.compile()
    _cache["nc"] = nc
    return nc


def _prep_w(weights_int8, scales):
    """[64, 4096] int8-valued weights -> [128, HT*EXPERTS] int8 with
    w_arr[p, k*64 + e] = W[e, 128k + p] (cast to fp16 on device)."""
    wt = weights_int8.astype(np.int8).T                        # [H, E]
    arr = wt.reshape(HT, 128, EXPERTS).transpose(1, 0, 2)
    return np.ascontiguousarray(arr).reshape(128, HT * EXPERTS)


def _prep_x(x):
    """Quantize + transpose x into per-core (x8, xi) plus token scales:
    x8[p, s*8+k, t]   = fp8((x[T0 + s*1024 + t, 128k + p]) / s_tok)   k<8
    xi[p, s*24+k', t] = rint(x[T0 + s*1024 + t, 128(k'+8) + p] / s_tok)
    """
    f8np = mybir.dt.np(F8)
    s_tok = np.abs(x).max(axis=1) / 127.0            # [TOKENS]
    s_tok = np.maximum(s_tok, 1e-12).astype(np.float32)
    xs = x / s_tok[:, None]                          # |xs| <= 127
    H8 = HT8 * 128
    x8 = xs[:, :H8].astype(f8np)
    xi = np.clip(np.rint(xs[:, H8:]), -127, 127).astype(np.int8)
    xt8 = np.empty((H8, TOKENS), dtype=f8np)
    xti = np.empty((HIDDEN - H8, TOKENS), dtype=np.int8)
    blk = 512
    for i in range(0, TOKENS, blk):
        xt8[:, i:i + blk] = x8[i:i + blk].T
        xti[:, i:i + blk] = xi[i:i + blk].T
    shards = []
    for c in range(NCORES):
        sl = slice(c * TSHARD, (c + 1) * TSHARD)
        # [H=k*128, 2048=NS*SU] -> [128, NS, k, SU]
        a8 = xt8[:, sl].reshape(HT8, 128, NS, SU).transpose(1, 2, 0, 3)
        ai = xti[:, sl].reshape(HTI, 128, NS, SU).transpose(1, 2, 0, 3)
        shards.append((
            np.ascontiguousarray(a8).reshape(128, NS * HT8, SU),
            np.ascontiguousarray(ai).reshape(128, NS * HTI, SU),
        ))
    return shards, s_tok


def kernel(x, weights_int8, scales):
    nc = _build()
    x = np.ascontiguousarray(np.asarray(x), dtype=np.float32)
    warr = _prep_w(np.asarray(weights_int8), np.asarray(scales))
    shards, s_tok = _prep_x(x)
    in_maps = [{"x8": shards[c][0], "xi": shards[c][1], "w": warr}
               for c in range(NCORES)]
    res = run_bass_kernel_spmd(nc, in_maps, core_ids=list(range(NCORES)))
    scales_f = np.asarray(scales, dtype=np.float64)
    out = np.empty((TOKENS, EXPERTS), dtype=np.float64)
    for c in range(NCORES):
        o = res.results[c]["out"].astype(np.float64)      # [128, NS*CH]
        o = o.reshape(2, 64, NS, CH)                      # [chunk, e, s, t]
        for s in range(NS):
            for ch in range(2):
                t0 = c * TSHARD + s * SU + ch * CH
                out[t0:t0 + CH] = o[ch, :, s, :].T
    out *= 64.0 * s_tok[:, None].astype(np.float64)
    out *= scales_f[None, :]
    return np.ascontiguousarray(out, dtype=np.float32)
